# revision 9
# baseline (speedup 1.0000x reference)
"""Bidirectional attention kernel for Trainium2 (8 NeuronCores, batch-parallel).

Math (per batch element, all on one core):
    k1p = k1 @ W1 + b1            [N, A]
    k2p = k2 @ W2 + b2            [N, A]
    S   = k1p @ k2p.T             [N, N]
    E   = exp(S)                  (no max-subtraction needed: |S| < ~25)
    o1[m, d] = sum_n E[n, m] v1[n, d] / sum_n E[n, m]   (softmax over N1)
    o2[n, d] = sum_m E[n, m] v2[m, d] / sum_m E[n, m]   (softmax over N2)

Both softmaxes share the same unnormalized exp(S); the normalizers are folded
into the output matmuls by appending a ones-column to v1/v2 and dividing the
PSUM result by its last column (per-partition scalar).  E is produced in both
orientations (E and E^T) since each output matmul needs its contraction dim on
partitions.  Projection/score matmuls run in float32r (full PE rate at free
dim >= 256); output matmuls run in bf16 with fp32 PSUM accumulation.
"""

import numpy as np

import concourse.bass as bass
import concourse.tile as tile
from concourse import bacc, mybir, bass_utils
from concourse.masks import make_identity

N_CORES = 8
B = 8
N = 2048  # N1 == N2
KD = 256  # K1D == K2D
VD = 256  # V1D == V2D
AD = 128
P = 128

F32 = mybir.dt.float32
F32R = mybir.dt.float32r
BF16 = mybir.dt.bfloat16
AF = mybir.ActivationFunctionType


def _emit_body_pools(nc, tc, consts, persist, pools, dram, n, skip_score=False, skip_o=False):
    """Emit one full pass of the kernel body using caller-provided pools
    (used by the For_i timing variant, where pools must live outside the
    loop)."""
    nt = n // P
    nch = n // 512
    k1_d, k2_d, v1_d, v2_d, o1_d, o2_d = (
        dram["k1"],
        dram["k2"],
        dram["v1"],
        dram["v2"],
        dram["o1"],
        dram["o2"],
    )
    identity, W1_sb, b1_sb, W2_sb, b2_sb = consts
    stage, ktbuf_pool, ptrans, pproj, pscore, po_pool, osb_pool, rc_pool = pools

    k1pT = persist.tile([P, n], F32R, tag="k1pT", name="k1pT")
    k2pT = persist.tile([P, n], F32R, tag="k2pT", name="k2pT")
    E = persist.tile([P, nt, n], BF16, tag="E", name="E")
    ET = persist.tile([P, nt, n], BF16, tag="ET", name="ET")
    v1e = persist.tile([P, nt, VD + 2], BF16, tag="v1e", name="v1e")
    v2e = persist.tile([P, nt, VD + 2], BF16, tag="v2e", name="v2e")

    for k_d, W_sb, b_sb, kpT in (
        (k1_d, W1_sb, b1_sb, k1pT),
        (k2_d, W2_sb, b2_sb, k2pT),
    ):
        for c in range(nch):
            st = stage.tile([P, 4, KD], F32, tag="stage", name="st")
            nc.sync.dma_start(
                out=st,
                in_=k_d[512 * c : 512 * (c + 1), :].rearrange("(t p) k -> p t k", p=P),
            )
            kt = ktbuf_pool.tile([P, 2, 512], F32R, tag="kt", name="kt")
            for kb in range(2):
                pt = ptrans.tile([P, 512], F32, tag="pt512", name="pt")
                for t in range(4):
                    nc.tensor.transpose(
                        pt[:, 128 * t : 128 * (t + 1)],
                        st[:, t, 128 * kb : 128 * (kb + 1)],
                        identity,
                    )
                nc.vector.tensor_copy(kt[:, kb, :], pt)
            pp = pproj.tile([P, 512], F32, tag="pt512", name="pp")
            for kb in range(2):
                nc.tensor.matmul(
                    pp,
                    lhsT=W_sb[:, kb, :],
                    rhs=kt[:, kb, :],
                    start=(kb == 0),
                    stop=(kb == 1),
                )
            nc.scalar.activation(
                kpT[:, 512 * c : 512 * (c + 1)], pp, AF.Identity, bias=b_sb, scale=1.0
            )
    for v_d, ve in ((v1_d, v1e), (v2_d, v2e)):
        nc.vector.memset(ve[:, :, VD : VD + 2], 1.0)
        for c in range(nch):
            sv = stage.tile([P, 4, VD], F32, tag="stage", name="sv")
            nc.sync.dma_start(
                out=sv,
                in_=v_d[512 * c : 512 * (c + 1), :].rearrange("(t p) d -> p t d", p=P),
            )
            nc.any.tensor_copy(ve[:, 4 * c : 4 * (c + 1), 0:VD], sv)

    w = min(1024, n)
    if not skip_score:
        for lhs_src, rhs_src, Edst in ((k1pT, k2pT, E), (k2pT, k1pT, ET)):
            for i in range(nt):
                for h in range(n // w):
                    ps = pscore.tile([P, w], F32, tag="ps", name="ps")
                    for q in range(w // 512):
                        col = w * h + 512 * q
                        nc.tensor.matmul(
                            ps[:, 512 * q : 512 * (q + 1)],
                            lhsT=lhs_src[:, 128 * i : 128 * (i + 1)],
                            rhs=rhs_src[:, col : col + 512],
                            start=True,
                            stop=True,
                        )
                    nc.scalar.activation(Edst[:, i, w * h : w * (h + 1)], ps, AF.Exp)
    else:
        # touch E/ET so later readers have a writer
        nc.vector.memset(E[:, 0, 0:2], 1.0)
        nc.vector.memset(ET[:, 0, 0:2], 1.0)

    if skip_o:
        return
    for Esrc, ve, o_d in ((E, v1e, o1_d), (ET, v2e, o2_d)):
        for mt in range(nt):
            pot = po_pool.tile([P, VD + 1], F32, tag="po", name="pot")
            for j in range(nt):
                nc.tensor.matmul(
                    pot,
                    lhsT=Esrc[:, j, 128 * mt : 128 * (mt + 1)],
                    rhs=ve[:, j, 0 : VD + 1],
                    start=(j == 0),
                    stop=(j == nt - 1),
                )
            rc = rc_pool.tile([P, 1], F32, tag="rc", name="rct")
            nc.vector.reciprocal(rc, pot[:, VD : VD + 1])
            ob = osb_pool.tile([P, VD], F32, tag="ob", name="ob")
            nc.vector.tensor_scalar_mul(ob, pot[:, 0:VD], rc)
            nc.sync.dma_start(out=o_d[128 * mt : 128 * (mt + 1), :], in_=ob)


def build_nc_loop(n: int = N, iters: int = 16, skip_score=False, skip_o=False):
    """Timing variant: whole body inside a hardware For_i loop."""
    nc = bacc.Bacc("TRN2", target_bir_lowering=False, debug=False)
    dram = {
        "k1": nc.dram_tensor("k1", [n, KD], F32, kind="ExternalInput").ap(),
        "k2": nc.dram_tensor("k2", [n, KD], F32, kind="ExternalInput").ap(),
        "v1": nc.dram_tensor("v1", [n, VD], F32, kind="ExternalInput").ap(),
        "v2": nc.dram_tensor("v2", [n, VD], F32, kind="ExternalInput").ap(),
        "o1": nc.dram_tensor("o1", [n, VD], F32, kind="ExternalOutput").ap(),
        "o2": nc.dram_tensor("o2", [n, VD], F32, kind="ExternalOutput").ap(),
    }
    W1_d = nc.dram_tensor("W1", [KD, AD], F32R, kind="ExternalInput").ap()
    b1_d = nc.dram_tensor("b1", [AD], F32, kind="ExternalInput").ap()
    W2_d = nc.dram_tensor("W2", [KD, AD], F32R, kind="ExternalInput").ap()
    b2_d = nc.dram_tensor("b2", [AD], F32, kind="ExternalInput").ap()

    with tile.TileContext(nc) as tc:
        with tc.tile_pool(name="consts", bufs=1) as consts_pool, tc.tile_pool(
            name="persist", bufs=1
        ) as persist, tc.tile_pool(name="stage", bufs=3) as stage, tc.tile_pool(
            name="ktbuf", bufs=2
        ) as ktbuf_pool, tc.tile_pool(
            name="ptpp", bufs=1, space="PSUM"
        ) as ptpp, tc.tile_pool(
            name="pscore", bufs=2, space="PSUM"
        ) as pscore, tc.tile_pool(
            name="po", bufs=3, space="PSUM"
        ) as po_pool, tc.tile_pool(name="osb", bufs=4) as osb_pool, tc.tile_pool(
            name="rc", bufs=4
        ) as rc_pool:
            identity = consts_pool.tile([P, P], F32)
            make_identity(nc, identity)
            W1_sb = consts_pool.tile([P, 2, AD], F32R)
            nc.sync.dma_start(out=W1_sb, in_=W1_d.rearrange("(kb k) a -> k kb a", k=P))
            W2_sb = consts_pool.tile([P, 2, AD], F32R)
            nc.sync.dma_start(out=W2_sb, in_=W2_d.rearrange("(kb k) a -> k kb a", k=P))
            b1_sb = consts_pool.tile([P, 1], F32)
            nc.sync.dma_start(out=b1_sb, in_=b1_d.rearrange("(a one) -> a one", one=1))
            b2_sb = consts_pool.tile([P, 1], F32)
            nc.sync.dma_start(out=b2_sb, in_=b2_d.rearrange("(a one) -> a one", one=1))
            consts = (identity, W1_sb, b1_sb, W2_sb, b2_sb)
            pools = (stage, ktbuf_pool, ptpp, ptpp, pscore, po_pool, osb_pool, rc_pool)

            with tc.For_i(0, iters, 1):
                _emit_body_pools(nc, tc, consts, persist, pools, dram, n,
                                 skip_score=skip_score, skip_o=skip_o)

    nc.compile()
    return nc


def _emit_body(nc, tc, consts, persist, dram, n):
    """Emit one full pass of the kernel body."""
    nt = n // P
    nch = n // 512
    k1_d, k2_d, v1_d, v2_d, o1_d, o2_d = (
        dram["k1"],
        dram["k2"],
        dram["v1"],
        dram["v2"],
        dram["o1"],
        dram["o2"],
    )
    identity, W1_sb, b1_sb, W2_sb, b2_sb = consts

    # Persistent SBUF tensors (same tag every rep -> same slots, serialized).
    k1pT = persist.tile([P, n], F32R, tag="k1pT")  # [a, n] projected k1^T
    k2pT = persist.tile([P, n], F32R, tag="k2pT")  # [a, m]
    E = persist.tile([P, nt, n], BF16, tag="E")  # E[p,i,m] = exp(S[128i+p, m])
    ET = persist.tile([P, nt, n], BF16, tag="ET")  # ET[p,i,n] = exp(S[n, 128i+p])
    v1e = persist.tile([P, nt, VD + 2], BF16, tag="v1e")  # bf16 v1 + ones col
    v2e = persist.tile([P, nt, VD + 2], BF16, tag="v2e")

    # ---- Phase 1: load, transpose, project; build v1e/v2e ----
    with tc.tile_pool(name="stage", bufs=3) as stage, tc.tile_pool(
        name="ktbuf", bufs=2
    ) as ktbuf_pool, tc.tile_pool(
        name="ptrans", bufs=2, space="PSUM"
    ) as ptrans, tc.tile_pool(name="pproj", bufs=2, space="PSUM") as pproj:
        for k_d, W_sb, b_sb, kpT in (
            (k1_d, W1_sb, b1_sb, k1pT),
            (k2_d, W2_sb, b2_sb, k2pT),
        ):
            for c in range(nch):
                st = stage.tile([P, 4, KD], F32, tag="stage")
                nc.sync.dma_start(
                    out=st,
                    in_=k_d[512 * c : 512 * (c + 1), :].rearrange(
                        "(t p) k -> p t k", p=P
                    ),
                )
                kt = ktbuf_pool.tile([P, 2, 512], F32R, tag="kt")
                for kb in range(2):
                    pt = ptrans.tile([P, 512], F32, tag="pt")
                    for t in range(4):
                        nc.tensor.transpose(
                            pt[:, 128 * t : 128 * (t + 1)],
                            st[:, t, 128 * kb : 128 * (kb + 1)],
                            identity,
                        )
                    nc.vector.tensor_copy(kt[:, kb, :], pt)
                pp = pproj.tile([P, 512], F32, tag="pp")
                for kb in range(2):
                    nc.tensor.matmul(
                        pp,
                        lhsT=W_sb[:, kb, :],
                        rhs=kt[:, kb, :],
                        start=(kb == 0),
                        stop=(kb == 1),
                    )
                nc.scalar.activation(
                    kpT[:, 512 * c : 512 * (c + 1)],
                    pp,
                    AF.Identity,
                    bias=b_sb,
                    scale=1.0,
                )
        for v_d, ve in ((v1_d, v1e), (v2_d, v2e)):
            nc.vector.memset(ve[:, :, VD : VD + 2], 1.0)
            for c in range(nch):
                sv = stage.tile([P, 4, VD], F32, tag="stage")
                nc.sync.dma_start(
                    out=sv,
                    in_=v_d[512 * c : 512 * (c + 1), :].rearrange(
                        "(t p) d -> p t d", p=P
                    ),
                )
                nc.any.tensor_copy(ve[:, 4 * c : 4 * (c + 1), 0:VD], sv)

    # ---- Phase 2: score matmuls + exp (both orientations) ----
    # ---- Phase 3: output matmuls with folded softmax denominators ----
    with tc.tile_pool(name="pscore", bufs=2, space="PSUM") as pscore, (
        tc.tile_pool(name="po", bufs=4, space="PSUM")
    ) as po_pool, tc.tile_pool(name="osb", bufs=4) as osb_pool, tc.tile_pool(
        name="rc", bufs=4
    ) as rc_pool:
        w = min(1024, n)
        for lhs_src, rhs_src, Edst in ((k1pT, k2pT, E), (k2pT, k1pT, ET)):
            for i in range(nt):
                for h in range(n // w):
                    ps = pscore.tile([P, w], F32, tag="ps")
                    for q in range(w // 512):
                        col = w * h + 512 * q
                        nc.tensor.matmul(
                            ps[:, 512 * q : 512 * (q + 1)],
                            lhsT=lhs_src[:, 128 * i : 128 * (i + 1)],
                            rhs=rhs_src[:, col : col + 512],
                            start=True,
                            stop=True,
                        )
                    nc.scalar.activation(
                        Edst[:, i, w * h : w * (h + 1)], ps, AF.Exp
                    )

        for Esrc, ve, o_d in ((E, v1e, o1_d), (ET, v2e, o2_d)):
            for mt in range(nt):
                pot = po_pool.tile([P, VD + 1], F32, tag="po")
                for j in range(nt):
                    nc.tensor.matmul(
                        pot,
                        lhsT=Esrc[:, j, 128 * mt : 128 * (mt + 1)],
                        rhs=ve[:, j, 0 : VD + 1],
                        start=(j == 0),
                        stop=(j == nt - 1),
                    )
                rc = rc_pool.tile([P, 1], F32, tag="rc")
                nc.vector.reciprocal(rc, pot[:, VD : VD + 1])
                ob = osb_pool.tile([P, VD], F32, tag="ob")
                nc.vector.tensor_scalar_mul(ob, pot[:, 0:VD], rc)
                nc.sync.dma_start(out=o_d[128 * mt : 128 * (mt + 1), :], in_=ob)


def build_nc(n: int = N, reps: int = 1):
    """Build the single-core SPMD program. `n` is the row count (2048 on HW;
    smaller for simulator checks). `reps` repeats the body for timing runs."""
    nc = bacc.Bacc("TRN2", target_bir_lowering=False, debug=False)

    dram = {
        "k1": nc.dram_tensor("k1", [n, KD], F32, kind="ExternalInput").ap(),
        "k2": nc.dram_tensor("k2", [n, KD], F32, kind="ExternalInput").ap(),
        "v1": nc.dram_tensor("v1", [n, VD], F32, kind="ExternalInput").ap(),
        "v2": nc.dram_tensor("v2", [n, VD], F32, kind="ExternalInput").ap(),
        "o1": nc.dram_tensor("o1", [n, VD], F32, kind="ExternalOutput").ap(),
        "o2": nc.dram_tensor("o2", [n, VD], F32, kind="ExternalOutput").ap(),
    }
    W1_d = nc.dram_tensor("W1", [KD, AD], F32R, kind="ExternalInput").ap()
    b1_d = nc.dram_tensor("b1", [AD], F32, kind="ExternalInput").ap()
    W2_d = nc.dram_tensor("W2", [KD, AD], F32R, kind="ExternalInput").ap()
    b2_d = nc.dram_tensor("b2", [AD], F32, kind="ExternalInput").ap()

    with tile.TileContext(nc) as tc:
        with tc.tile_pool(name="consts", bufs=1) as consts_pool, tc.tile_pool(
            name="persist", bufs=1
        ) as persist:
            identity = consts_pool.tile([P, P], F32)
            make_identity(nc, identity)
            W1_sb = consts_pool.tile([P, 2, AD], F32R)
            nc.sync.dma_start(out=W1_sb, in_=W1_d.rearrange("(kb k) a -> k kb a", k=P))
            W2_sb = consts_pool.tile([P, 2, AD], F32R)
            nc.sync.dma_start(out=W2_sb, in_=W2_d.rearrange("(kb k) a -> k kb a", k=P))
            b1_sb = consts_pool.tile([P, 1], F32)
            nc.sync.dma_start(out=b1_sb, in_=b1_d.rearrange("(a one) -> a one", one=1))
            b2_sb = consts_pool.tile([P, 1], F32)
            nc.sync.dma_start(out=b2_sb, in_=b2_d.rearrange("(a one) -> a one", one=1))
            consts = (identity, W1_sb, b1_sb, W2_sb, b2_sb)

            for _ in range(reps):
                _emit_body(nc, tc, consts, persist, dram, n)

    nc.compile()
    return nc


_NC_CACHE: dict = {}


def _get_nc(n: int = N):
    if n not in _NC_CACHE:
        _NC_CACHE[n] = build_nc(n)
    return _NC_CACHE[n]


def kernel(k1, k2, v1, v2, W1, b1, W2, b2):
    """Full-input entry point: shard batch across 8 cores, run SPMD, gather."""
    nc = _get_nc(N)
    k1 = np.ascontiguousarray(np.asarray(k1, dtype=np.float32))
    k2 = np.ascontiguousarray(np.asarray(k2, dtype=np.float32))
    v1 = np.ascontiguousarray(np.asarray(v1, dtype=np.float32))
    v2 = np.ascontiguousarray(np.asarray(v2, dtype=np.float32))
    W1 = np.ascontiguousarray(np.asarray(W1, dtype=np.float32))
    b1 = np.ascontiguousarray(np.asarray(b1, dtype=np.float32))
    W2 = np.ascontiguousarray(np.asarray(W2, dtype=np.float32))
    b2 = np.ascontiguousarray(np.asarray(b2, dtype=np.float32))
    in_maps = [
        {
            "k1": k1[c],
            "k2": k2[c],
            "v1": v1[c],
            "v2": v2[c],
            "W1": W1,
            "b1": b1,
            "W2": W2,
            "b2": b2,
        }
        for c in range(N_CORES)
    ]
    res = bass_utils.run_bass_kernel_spmd(nc, in_maps, core_ids=list(range(N_CORES)))
    o2 = np.stack([res.results[c]["o2"] for c in range(N_CORES)])
    o1 = np.stack([res.results[c]["o1"] for c in range(N_CORES)])
    return (o2, o1)


# revision 10
# speedup vs baseline: 1.1166x; 1.1166x over previous
"""Bidirectional attention kernel for Trainium2 (8 NeuronCores, batch-parallel).

Math (per batch element, all on one core):
    k1p = k1 @ W1 + b1            [N, A]
    k2p = k2 @ W2 + b2            [N, A]
    S   = k1p @ k2p.T             [N, N]
    E   = exp(S)                  (no max-subtraction needed: |S| < ~25)
    o1[m, d] = sum_n E[n, m] v1[n, d] / sum_n E[n, m]   (softmax over N1)
    o2[n, d] = sum_m E[n, m] v2[m, d] / sum_m E[n, m]   (softmax over N2)

Both softmaxes share the same unnormalized exp(S); the normalizers are folded
into the output matmuls by appending a ones-column to v1/v2 and dividing the
PSUM result by its last column (per-partition scalar).  E is produced in both
orientations (E and E^T) since each output matmul needs its contraction dim on
partitions.  Projection/score matmuls run in float32r (full PE rate at free
dim >= 256); output matmuls run in bf16 with fp32 PSUM accumulation.
"""

import numpy as np

import concourse.bass as bass
import concourse.tile as tile
from concourse import bacc, mybir, bass_utils
from concourse.masks import make_identity

N_CORES = 8
B = 8
N = 2048  # N1 == N2
KD = 256  # K1D == K2D
VD = 256  # V1D == V2D
AD = 128
P = 128

F32 = mybir.dt.float32
F32R = mybir.dt.float32r
BF16 = mybir.dt.bfloat16
AF = mybir.ActivationFunctionType


def _emit_body_pools(nc, tc, consts, persist, pools, dram, n, skip_score=False, skip_o=False):
    """Emit one full pass of the kernel body using caller-provided pools
    (used by the For_i timing variant, where pools must live outside the
    loop)."""
    nt = n // P
    nch = n // 512
    k1_d, k2_d, v1_d, v2_d, o1_d, o2_d = (
        dram["k1"],
        dram["k2"],
        dram["v1"],
        dram["v2"],
        dram["o1"],
        dram["o2"],
    )
    identity, W1_sb, b1_sb, W2_sb, b2_sb = consts
    stage, ktbuf_pool, ptrans, pproj, pscore, po_pool, osb_pool, rc_pool = pools

    k1pT = persist.tile([P, n], F32R, tag="k1pT", name="k1pT")
    k2pT = persist.tile([P, n], F32R, tag="k2pT", name="k2pT")
    E = persist.tile([P, nt, n], BF16, tag="E", name="E")
    ET = persist.tile([P, nt, n], BF16, tag="ET", name="ET")
    v1e = persist.tile([P, nt, VD + 2], BF16, tag="v1e", name="v1e")
    v2e = persist.tile([P, nt, VD + 2], BF16, tag="v2e", name="v2e")

    for k_d, W_sb, b_sb, kpT in (
        (k1_d, W1_sb, b1_sb, k1pT),
        (k2_d, W2_sb, b2_sb, k2pT),
    ):
        for c in range(nch):
            st = stage.tile([P, 4, KD], F32, tag="stage", name="st")
            nc.sync.dma_start(
                out=st,
                in_=k_d[512 * c : 512 * (c + 1), :].rearrange("(t p) k -> p t k", p=P),
            )
            kt = ktbuf_pool.tile([P, 2, 512], F32R, tag="kt", name="kt")
            for kb in range(2):
                pt = ptrans.tile([P, 512], F32, tag="pt512", name="pt")
                for t in range(4):
                    nc.tensor.transpose(
                        pt[:, 128 * t : 128 * (t + 1)],
                        st[:, t, 128 * kb : 128 * (kb + 1)],
                        identity,
                    )
                nc.any.tensor_copy(kt[:, kb, :], pt)
            pp = pproj.tile([P, 512], F32, tag="pt512", name="pp")
            for kb in range(2):
                nc.tensor.matmul(
                    pp,
                    lhsT=W_sb[:, kb, :],
                    rhs=kt[:, kb, :],
                    start=(kb == 0),
                    stop=(kb == 1),
                )
            nc.scalar.activation(
                kpT[:, 512 * c : 512 * (c + 1)], pp, AF.Identity, bias=b_sb, scale=1.0
            )
    for v_d, ve in ((v1_d, v1e), (v2_d, v2e)):
        nc.vector.memset(ve[:, :, VD : VD + 2], 1.0)
        for c in range(nch):
            sv = stage.tile([P, 4, VD], F32, tag="stage", name="sv")
            nc.sync.dma_start(
                out=sv,
                in_=v_d[512 * c : 512 * (c + 1), :].rearrange("(t p) d -> p t d", p=P),
            )
            nc.any.tensor_copy(ve[:, 4 * c : 4 * (c + 1), 0:VD], sv)

    w = min(1024, n)
    if not skip_score:
        for lhs_src, rhs_src, Edst in ((k1pT, k2pT, E), (k2pT, k1pT, ET)):
            for i in range(nt):
                for h in range(n // w):
                    ps = pscore.tile([P, w], F32, tag="ps", name="ps")
                    for q in range(w // 512):
                        col = w * h + 512 * q
                        nc.tensor.matmul(
                            ps[:, 512 * q : 512 * (q + 1)],
                            lhsT=lhs_src[:, 128 * i : 128 * (i + 1)],
                            rhs=rhs_src[:, col : col + 512],
                            start=True,
                            stop=True,
                        )
                    nc.scalar.activation(Edst[:, i, w * h : w * (h + 1)], ps, AF.Exp)
    else:
        # touch E/ET so later readers have a writer
        nc.vector.memset(E[:, 0, 0:2], 1.0)
        nc.vector.memset(ET[:, 0, 0:2], 1.0)

    if skip_o:
        return
    for Esrc, ve, o_d in ((E, v1e, o1_d), (ET, v2e, o2_d)):
        for mt in range(nt):
            pot = po_pool.tile([P, VD + 1], F32, tag="po", name="pot")
            for j in range(nt):
                nc.tensor.matmul(
                    pot,
                    lhsT=Esrc[:, j, 128 * mt : 128 * (mt + 1)],
                    rhs=ve[:, j, 0 : VD + 1],
                    start=(j == 0),
                    stop=(j == nt - 1),
                )
            rc = rc_pool.tile([P, 1], F32, tag="rc", name="rct")
            nc.vector.reciprocal(rc, pot[:, VD : VD + 1])
            ob = osb_pool.tile([P, VD], F32, tag="ob", name="ob")
            nc.vector.tensor_scalar_mul(ob, pot[:, 0:VD], rc)
            nc.sync.dma_start(out=o_d[128 * mt : 128 * (mt + 1), :], in_=ob)


def build_nc_loop(n: int = N, iters: int = 16, skip_score=False, skip_o=False):
    """Timing variant: whole body inside a hardware For_i loop."""
    nc = bacc.Bacc("TRN2", target_bir_lowering=False, debug=False)
    dram = {
        "k1": nc.dram_tensor("k1", [n, KD], F32, kind="ExternalInput").ap(),
        "k2": nc.dram_tensor("k2", [n, KD], F32, kind="ExternalInput").ap(),
        "v1": nc.dram_tensor("v1", [n, VD], F32, kind="ExternalInput").ap(),
        "v2": nc.dram_tensor("v2", [n, VD], F32, kind="ExternalInput").ap(),
        "o1": nc.dram_tensor("o1", [n, VD], F32, kind="ExternalOutput").ap(),
        "o2": nc.dram_tensor("o2", [n, VD], F32, kind="ExternalOutput").ap(),
    }
    W1_d = nc.dram_tensor("W1", [KD, AD], F32R, kind="ExternalInput").ap()
    b1_d = nc.dram_tensor("b1", [AD], F32, kind="ExternalInput").ap()
    W2_d = nc.dram_tensor("W2", [KD, AD], F32R, kind="ExternalInput").ap()
    b2_d = nc.dram_tensor("b2", [AD], F32, kind="ExternalInput").ap()

    with tile.TileContext(nc) as tc:
        with tc.tile_pool(name="consts", bufs=1) as consts_pool, tc.tile_pool(
            name="persist", bufs=1
        ) as persist, tc.tile_pool(name="stage", bufs=3) as stage, tc.tile_pool(
            name="ktbuf", bufs=2
        ) as ktbuf_pool, tc.tile_pool(
            name="ptpp", bufs=2, space="PSUM"
        ) as ptpp, tc.tile_pool(
            name="pscore", bufs=2, space="PSUM"
        ) as pscore, tc.tile_pool(
            name="po", bufs=2, space="PSUM"
        ) as po_pool, tc.tile_pool(name="osb", bufs=4) as osb_pool, tc.tile_pool(
            name="rc", bufs=4
        ) as rc_pool:
            identity = consts_pool.tile([P, P], F32)
            make_identity(nc, identity)
            W1_sb = consts_pool.tile([P, 2, AD], F32R)
            nc.sync.dma_start(out=W1_sb, in_=W1_d.rearrange("(kb k) a -> k kb a", k=P))
            W2_sb = consts_pool.tile([P, 2, AD], F32R)
            nc.sync.dma_start(out=W2_sb, in_=W2_d.rearrange("(kb k) a -> k kb a", k=P))
            b1_sb = consts_pool.tile([P, 1], F32)
            nc.sync.dma_start(out=b1_sb, in_=b1_d.rearrange("(a one) -> a one", one=1))
            b2_sb = consts_pool.tile([P, 1], F32)
            nc.sync.dma_start(out=b2_sb, in_=b2_d.rearrange("(a one) -> a one", one=1))
            consts = (identity, W1_sb, b1_sb, W2_sb, b2_sb)
            pools = (stage, ktbuf_pool, ptpp, ptpp, pscore, po_pool, osb_pool, rc_pool)

            with tc.For_i(0, iters, 1):
                _emit_body_pools(nc, tc, consts, persist, pools, dram, n,
                                 skip_score=skip_score, skip_o=skip_o)

    nc.compile()
    return nc


def _emit_body(nc, tc, consts, persist, dram, n):
    """Emit one full pass of the kernel body."""
    nt = n // P
    nch = n // 512
    k1_d, k2_d, v1_d, v2_d, o1_d, o2_d = (
        dram["k1"],
        dram["k2"],
        dram["v1"],
        dram["v2"],
        dram["o1"],
        dram["o2"],
    )
    identity, W1_sb, b1_sb, W2_sb, b2_sb = consts

    # Persistent SBUF tensors (same tag every rep -> same slots, serialized).
    k1pT = persist.tile([P, n], F32R, tag="k1pT")  # [a, n] projected k1^T
    k2pT = persist.tile([P, n], F32R, tag="k2pT")  # [a, m]
    E = persist.tile([P, nt, n], BF16, tag="E")  # E[p,i,m] = exp(S[128i+p, m])
    ET = persist.tile([P, nt, n], BF16, tag="ET")  # ET[p,i,n] = exp(S[n, 128i+p])
    v1e = persist.tile([P, nt, VD + 2], BF16, tag="v1e")  # bf16 v1 + ones col
    v2e = persist.tile([P, nt, VD + 2], BF16, tag="v2e")

    # ---- Phase 1: load, transpose, project; build v1e/v2e ----
    with tc.tile_pool(name="stage", bufs=3) as stage, tc.tile_pool(
        name="ktbuf", bufs=2
    ) as ktbuf_pool, tc.tile_pool(
        name="ptrans", bufs=2, space="PSUM"
    ) as ptrans, tc.tile_pool(name="pproj", bufs=2, space="PSUM") as pproj:
        for k_d, W_sb, b_sb, kpT in (
            (k1_d, W1_sb, b1_sb, k1pT),
            (k2_d, W2_sb, b2_sb, k2pT),
        ):
            for c in range(nch):
                st = stage.tile([P, 4, KD], F32, tag="stage")
                nc.sync.dma_start(
                    out=st,
                    in_=k_d[512 * c : 512 * (c + 1), :].rearrange(
                        "(t p) k -> p t k", p=P
                    ),
                )
                kt = ktbuf_pool.tile([P, 2, 512], F32R, tag="kt")
                for kb in range(2):
                    pt = ptrans.tile([P, 512], F32, tag="pt")
                    for t in range(4):
                        nc.tensor.transpose(
                            pt[:, 128 * t : 128 * (t + 1)],
                            st[:, t, 128 * kb : 128 * (kb + 1)],
                            identity,
                        )
                    nc.any.tensor_copy(kt[:, kb, :], pt)
                pp = pproj.tile([P, 512], F32, tag="pp")
                for kb in range(2):
                    nc.tensor.matmul(
                        pp,
                        lhsT=W_sb[:, kb, :],
                        rhs=kt[:, kb, :],
                        start=(kb == 0),
                        stop=(kb == 1),
                    )
                nc.scalar.activation(
                    kpT[:, 512 * c : 512 * (c + 1)],
                    pp,
                    AF.Identity,
                    bias=b_sb,
                    scale=1.0,
                )
        for v_d, ve in ((v1_d, v1e), (v2_d, v2e)):
            nc.vector.memset(ve[:, :, VD : VD + 2], 1.0)
            for c in range(nch):
                sv = stage.tile([P, 4, VD], F32, tag="stage")
                nc.sync.dma_start(
                    out=sv,
                    in_=v_d[512 * c : 512 * (c + 1), :].rearrange(
                        "(t p) d -> p t d", p=P
                    ),
                )
                nc.any.tensor_copy(ve[:, 4 * c : 4 * (c + 1), 0:VD], sv)

    # ---- Phase 2: score matmuls + exp (both orientations) ----
    # ---- Phase 3: output matmuls with folded softmax denominators ----
    with tc.tile_pool(name="pscore", bufs=2, space="PSUM") as pscore, (
        tc.tile_pool(name="po", bufs=4, space="PSUM")
    ) as po_pool, tc.tile_pool(name="osb", bufs=4) as osb_pool, tc.tile_pool(
        name="rc", bufs=4
    ) as rc_pool:
        w = min(1024, n)
        for lhs_src, rhs_src, Edst in ((k1pT, k2pT, E), (k2pT, k1pT, ET)):
            for i in range(nt):
                for h in range(n // w):
                    ps = pscore.tile([P, w], F32, tag="ps")
                    for q in range(w // 512):
                        col = w * h + 512 * q
                        nc.tensor.matmul(
                            ps[:, 512 * q : 512 * (q + 1)],
                            lhsT=lhs_src[:, 128 * i : 128 * (i + 1)],
                            rhs=rhs_src[:, col : col + 512],
                            start=True,
                            stop=True,
                        )
                    nc.scalar.activation(
                        Edst[:, i, w * h : w * (h + 1)], ps, AF.Exp
                    )

        for Esrc, ve, o_d in ((E, v1e, o1_d), (ET, v2e, o2_d)):
            for mt in range(nt):
                pot = po_pool.tile([P, VD + 1], F32, tag="po")
                for j in range(nt):
                    nc.tensor.matmul(
                        pot,
                        lhsT=Esrc[:, j, 128 * mt : 128 * (mt + 1)],
                        rhs=ve[:, j, 0 : VD + 1],
                        start=(j == 0),
                        stop=(j == nt - 1),
                    )
                rc = rc_pool.tile([P, 1], F32, tag="rc")
                nc.vector.reciprocal(rc, pot[:, VD : VD + 1])
                ob = osb_pool.tile([P, VD], F32, tag="ob")
                nc.vector.tensor_scalar_mul(ob, pot[:, 0:VD], rc)
                nc.sync.dma_start(out=o_d[128 * mt : 128 * (mt + 1), :], in_=ob)


def build_nc(n: int = N, reps: int = 1):
    """Build the single-core SPMD program. `n` is the row count (2048 on HW;
    smaller for simulator checks). `reps` repeats the body for timing runs."""
    nc = bacc.Bacc("TRN2", target_bir_lowering=False, debug=False)

    dram = {
        "k1": nc.dram_tensor("k1", [n, KD], F32, kind="ExternalInput").ap(),
        "k2": nc.dram_tensor("k2", [n, KD], F32, kind="ExternalInput").ap(),
        "v1": nc.dram_tensor("v1", [n, VD], F32, kind="ExternalInput").ap(),
        "v2": nc.dram_tensor("v2", [n, VD], F32, kind="ExternalInput").ap(),
        "o1": nc.dram_tensor("o1", [n, VD], F32, kind="ExternalOutput").ap(),
        "o2": nc.dram_tensor("o2", [n, VD], F32, kind="ExternalOutput").ap(),
    }
    W1_d = nc.dram_tensor("W1", [KD, AD], F32R, kind="ExternalInput").ap()
    b1_d = nc.dram_tensor("b1", [AD], F32, kind="ExternalInput").ap()
    W2_d = nc.dram_tensor("W2", [KD, AD], F32R, kind="ExternalInput").ap()
    b2_d = nc.dram_tensor("b2", [AD], F32, kind="ExternalInput").ap()

    with tile.TileContext(nc) as tc:
        with tc.tile_pool(name="consts", bufs=1) as consts_pool, tc.tile_pool(
            name="persist", bufs=1
        ) as persist:
            identity = consts_pool.tile([P, P], F32)
            make_identity(nc, identity)
            W1_sb = consts_pool.tile([P, 2, AD], F32R)
            nc.sync.dma_start(out=W1_sb, in_=W1_d.rearrange("(kb k) a -> k kb a", k=P))
            W2_sb = consts_pool.tile([P, 2, AD], F32R)
            nc.sync.dma_start(out=W2_sb, in_=W2_d.rearrange("(kb k) a -> k kb a", k=P))
            b1_sb = consts_pool.tile([P, 1], F32)
            nc.sync.dma_start(out=b1_sb, in_=b1_d.rearrange("(a one) -> a one", one=1))
            b2_sb = consts_pool.tile([P, 1], F32)
            nc.sync.dma_start(out=b2_sb, in_=b2_d.rearrange("(a one) -> a one", one=1))
            consts = (identity, W1_sb, b1_sb, W2_sb, b2_sb)

            for _ in range(reps):
                _emit_body(nc, tc, consts, persist, dram, n)

    nc.compile()
    return nc


_NC_CACHE: dict = {}


def _get_nc(n: int = N):
    if n not in _NC_CACHE:
        _NC_CACHE[n] = build_nc(n)
    return _NC_CACHE[n]


def kernel(k1, k2, v1, v2, W1, b1, W2, b2):
    """Full-input entry point: shard batch across 8 cores, run SPMD, gather."""
    nc = _get_nc(N)
    k1 = np.ascontiguousarray(np.asarray(k1, dtype=np.float32))
    k2 = np.ascontiguousarray(np.asarray(k2, dtype=np.float32))
    v1 = np.ascontiguousarray(np.asarray(v1, dtype=np.float32))
    v2 = np.ascontiguousarray(np.asarray(v2, dtype=np.float32))
    W1 = np.ascontiguousarray(np.asarray(W1, dtype=np.float32))
    b1 = np.ascontiguousarray(np.asarray(b1, dtype=np.float32))
    W2 = np.ascontiguousarray(np.asarray(W2, dtype=np.float32))
    b2 = np.ascontiguousarray(np.asarray(b2, dtype=np.float32))
    in_maps = [
        {
            "k1": k1[c],
            "k2": k2[c],
            "v1": v1[c],
            "v2": v2[c],
            "W1": W1,
            "b1": b1,
            "W2": W2,
            "b2": b2,
        }
        for c in range(N_CORES)
    ]
    res = bass_utils.run_bass_kernel_spmd(nc, in_maps, core_ids=list(range(N_CORES)))
    o2 = np.stack([res.results[c]["o2"] for c in range(N_CORES)])
    o1 = np.stack([res.results[c]["o1"] for c in range(N_CORES)])
    return (o2, o1)


# revision 17
# speedup vs baseline: 1.3146x; 1.1774x over previous
"""Bidirectional attention kernel for Trainium2 (8 NeuronCores, batch-parallel).

Math (per batch element, all on one core):
    k1p = k1 @ W1 + b1            [N, A]
    k2p = k2 @ W2 + b2            [N, A]
    S   = k1p @ k2p.T             [N, N]
    E   = exp(S)                  (no max-subtraction needed: |S| < ~25)
    o1[m, d] = sum_n E[n, m] v1[n, d] / sum_n E[n, m]   (softmax over N1)
    o2[n, d] = sum_m E[n, m] v2[m, d] / sum_m E[n, m]   (softmax over N2)

Both softmaxes share the same unnormalized exp(S); the normalizers are folded
into the output matmuls by appending a ones-column to v1/v2 and dividing the
PSUM result by its last column (per-partition scalar).  E is produced in both
orientations (E and E^T) since each output matmul needs its contraction dim on
partitions.  Projection/score matmuls run in float32r (full PE rate at free
dim >= 256); output matmuls run in bf16 with fp32 PSUM accumulation.
"""

import numpy as np

import concourse.bass as bass
import concourse.tile as tile
from concourse import bacc, mybir, bass_utils
from concourse.masks import make_identity

N_CORES = 8
B = 8
N = 2048  # N1 == N2
KD = 256  # K1D == K2D
VD = 256  # V1D == V2D
AD = 128
P = 128

F32 = mybir.dt.float32
F32R = mybir.dt.float32r
BF16 = mybir.dt.bfloat16
AF = mybir.ActivationFunctionType


def _emit_body_pools(nc, tc, consts, persist, pools, dram, n):
    """One body pass with caller-provided pools (For_i timing variant).
    Mirrors _emit_body; ptet shares the ptrans/pproj slots via tag."""
    nt = n // P
    nch = n // 512
    k1_d, k2_d, v1_d, v2_d, o1_d, o2_d = (
        dram["k1"], dram["k2"], dram["v1"], dram["v2"], dram["o1"], dram["o2"],
    )
    identity, id_bf, W1_sb, b1_sb, W2_sb, b2_sb = consts
    stage, ktbuf_pool, ptpp, pscore, po_pool, osb_pool, rc_pool = pools

    k1pT = persist.tile([P, n], F32R, tag="k1pT", name="k1pT")
    k2pT = persist.tile([P, n], F32R, tag="k2pT", name="k2pT")
    E = persist.tile([P, nt, n], BF16, tag="E", name="E")
    ET = persist.tile([P, nt, n], BF16, tag="ET", name="ET")
    v1e = persist.tile([P, nt, VD + 2], BF16, tag="v1e", name="v1e")
    v2e = persist.tile([P, nt, VD + 2], BF16, tag="v2e", name="v2e")

    for c in range(nch):
        for k_d, W_sb, b_sb, kpT in (
            (k1_d, W1_sb, b1_sb, k1pT),
            (k2_d, W2_sb, b2_sb, k2pT),
        ):
            st = stage.tile([P, 4, KD], F32, tag="stage", name="st")
            nc.sync.dma_start(
                out=st,
                in_=k_d[512 * c : 512 * (c + 1), :].rearrange("(t p) k -> p t k", p=P),
            )
            kt = ktbuf_pool.tile([P, 2, 512], F32R, tag="kt", name="kt")
            for kb in range(2):
                pt = ptpp.tile([P, 512], F32, tag="pt512", name="pt")
                for t in range(4):
                    nc.tensor.transpose(
                        pt[:, 128 * t : 128 * (t + 1)],
                        st[:, t, 128 * kb : 128 * (kb + 1)],
                        identity,
                    )
                nc.vector.tensor_copy(kt[:, kb, :], pt)
            pp = ptpp.tile([P, 512], F32, tag="pt512", name="pp")
            for kb in range(2):
                nc.tensor.matmul(
                    pp, lhsT=W_sb[:, kb, :], rhs=kt[:, kb, :],
                    start=(kb == 0), stop=(kb == 1),
                )
            nc.scalar.activation(
                kpT[:, 512 * c : 512 * (c + 1)], pp, AF.Identity, bias=b_sb, scale=1.0
            )
    for v_d, ve in ((v1_d, v1e), (v2_d, v2e)):
        nc.vector.memset(ve[:, :, VD : VD + 2], 1.0)
        for c in range(nch):
            sv = stage.tile([P, 4, VD], F32, tag="stage", name="sv")
            nc.sync.dma_start(
                out=sv,
                in_=v_d[512 * c : 512 * (c + 1), :].rearrange("(t p) d -> p t d", p=P),
            )
            nc.any.tensor_copy(ve[:, 4 * c : 4 * (c + 1), 0:VD], sv)

    w = min(1024, n)
    tg = min(4, nt)
    for i in range(nt):
        for h in range(n // w):
            ps = pscore.tile([P, w], F32, tag="ps", name="ps")
            for q in range(w // 512):
                col = w * h + 512 * q
                nc.tensor.matmul(
                    ps[:, 512 * q : 512 * (q + 1)],
                    lhsT=k1pT[:, 128 * i : 128 * (i + 1)],
                    rhs=k2pT[:, col : col + 512],
                    start=True,
                    stop=True,
                )
            nc.scalar.activation(E[:, i, w * h : w * (h + 1)], ps, AF.Exp)
        if i % tg == tg - 1:
            g = i // tg
            for j in range(nt):
                pt2 = ptpp.tile([P, 128 * tg], BF16, tag="pt512", name="pt2")
                for t in range(tg):
                    nc.tensor.transpose(
                        pt2[:, 128 * t : 128 * (t + 1)],
                        E[:, tg * g + t, 128 * j : 128 * (j + 1)],
                        id_bf,
                    )
                nc.any.tensor_copy(ET[:, j, 128 * tg * g : 128 * tg * (g + 1)], pt2)

    for mt in range(nt):
        _emit_o_group(nc, po_pool, rc_pool, osb_pool, E, v1e, o1_d, mt, nt)
    for mt in range(nt):
        _emit_o_group(nc, po_pool, rc_pool, osb_pool, ET, v2e, o2_d, mt, nt)


def build_nc_loop(n: int = N, iters: int = 16):
    """Timing variant: whole body inside a hardware For_i loop."""
    nc = bacc.Bacc("TRN2", target_bir_lowering=False, debug=False)
    dram = {
        "k1": nc.dram_tensor("k1", [n, KD], F32, kind="ExternalInput").ap(),
        "k2": nc.dram_tensor("k2", [n, KD], F32, kind="ExternalInput").ap(),
        "v1": nc.dram_tensor("v1", [n, VD], F32, kind="ExternalInput").ap(),
        "v2": nc.dram_tensor("v2", [n, VD], F32, kind="ExternalInput").ap(),
        "o1": nc.dram_tensor("o1", [n, VD], F32, kind="ExternalOutput").ap(),
        "o2": nc.dram_tensor("o2", [n, VD], F32, kind="ExternalOutput").ap(),
    }
    W1_d = nc.dram_tensor("W1", [KD, AD], F32R, kind="ExternalInput").ap()
    b1_d = nc.dram_tensor("b1", [AD], F32, kind="ExternalInput").ap()
    W2_d = nc.dram_tensor("W2", [KD, AD], F32R, kind="ExternalInput").ap()
    b2_d = nc.dram_tensor("b2", [AD], F32, kind="ExternalInput").ap()

    with tile.TileContext(nc) as tc:
        with tc.tile_pool(name="consts", bufs=1) as consts_pool, tc.tile_pool(
            name="persist", bufs=1
        ) as persist, tc.tile_pool(name="stage", bufs=5) as stage, tc.tile_pool(
            name="ktbuf", bufs=3
        ) as ktbuf_pool, tc.tile_pool(
            name="ptpp", bufs=2, space="PSUM"
        ) as ptpp, tc.tile_pool(
            name="pscore", bufs=2, space="PSUM"
        ) as pscore, tc.tile_pool(
            name="po", bufs=2, space="PSUM"
        ) as po_pool, tc.tile_pool(name="osb", bufs=4) as osb_pool, tc.tile_pool(
            name="rc", bufs=4
        ) as rc_pool:
            identity = consts_pool.tile([P, P], F32)
            make_identity(nc, identity)
            id_bf = consts_pool.tile([P, P], BF16)
            make_identity(nc, id_bf)
            W1_sb = consts_pool.tile([P, 2, AD], F32R)
            nc.sync.dma_start(out=W1_sb, in_=W1_d.rearrange("(kb k) a -> k kb a", k=P))
            W2_sb = consts_pool.tile([P, 2, AD], F32R)
            nc.sync.dma_start(out=W2_sb, in_=W2_d.rearrange("(kb k) a -> k kb a", k=P))
            b1_sb = consts_pool.tile([P, 1], F32)
            nc.sync.dma_start(out=b1_sb, in_=b1_d.rearrange("(a one) -> a one", one=1))
            b2_sb = consts_pool.tile([P, 1], F32)
            nc.sync.dma_start(out=b2_sb, in_=b2_d.rearrange("(a one) -> a one", one=1))
            consts = (identity, id_bf, W1_sb, b1_sb, W2_sb, b2_sb)
            pools = (stage, ktbuf_pool, ptpp, pscore, po_pool, osb_pool, rc_pool)

            with tc.For_i(0, iters, 1):
                _emit_body_pools(nc, tc, consts, persist, pools, dram, n)

    nc.compile()
    return nc


def _emit_o_group(nc, po_pool, rc_pool, osb_pool, Esrc, ve, o_d, mt, nt):
    """One output tile: 16-deep PSUM accumulation + folded-softmax normalize."""
    pot = po_pool.tile([P, VD + 1], F32, tag="po", name="pot")
    for j in range(nt):
        nc.tensor.matmul(
            pot,
            lhsT=Esrc[:, j, 128 * mt : 128 * (mt + 1)],
            rhs=ve[:, j, 0 : VD + 1],
            start=(j == 0),
            stop=(j == nt - 1),
        )
    rc = rc_pool.tile([P, 1], F32, tag="rc", name="rct")
    nc.vector.reciprocal(rc, pot[:, VD : VD + 1])
    ob = osb_pool.tile([P, VD], F32, tag="ob", name="ob")
    nc.scalar.activation(ob, pot[:, 0:VD], AF.Identity, scale=rc)
    nc.sync.dma_start(out=o_d[128 * mt : 128 * (mt + 1), :], in_=ob)


def _emit_body(nc, tc, consts, persist, dram, n, head=None):
    """Emit one full pass of the kernel body."""
    nt = n // P
    nch = n // 512
    k1_d, k2_d, v1_d, v2_d, o1_d, o2_d = (
        dram["k1"],
        dram["k2"],
        dram["v1"],
        dram["v2"],
        dram["o1"],
        dram["o2"],
    )
    identity, id_bf, W1_sb, b1_sb, W2_sb, b2_sb = consts

    # Persistent SBUF tensors (same tag every rep -> same slots, serialized).
    k1pT = persist.tile([P, n], F32R, tag="k1pT")  # [a, n] projected k1^T
    k2pT = persist.tile([P, n], F32R, tag="k2pT")  # [a, m]
    E = persist.tile([P, nt, n], BF16, tag="E")  # E[p,i,m] = exp(S[128i+p, m])
    ET = persist.tile([P, nt, n], BF16, tag="ET")  # ET[p,i,n] = exp(S[n, 128i+p])
    v1e = persist.tile([P, nt, VD + 2], BF16, tag="v1e")  # bf16 v1 + ones col
    v2e = persist.tile([P, nt, VD + 2], BF16, tag="v2e")

    # ---- Phase 1: load, transpose, project; build v1e/v2e ----
    with tc.tile_pool(name="stage", bufs=5) as stage, tc.tile_pool(
        name="ktbuf", bufs=3
    ) as ktbuf_pool, tc.tile_pool(
        name="ptrans", bufs=2, space="PSUM"
    ) as ptrans, tc.tile_pool(name="pproj", bufs=2, space="PSUM") as pproj:
        for c in range(nch):
            for k_d, W_sb, b_sb, kpT in (
                (k1_d, W1_sb, b1_sb, k1pT),
                (k2_d, W2_sb, b2_sb, k2pT),
            ):
                st = stage.tile([P, 4, KD], F32, tag="stage")
                use_head = c == 0 and k_d is k1_d and head is not None
                if use_head:
                    for t in range(2, 4):
                        nc.sync.dma_start(
                            out=st[:, t, :],
                            in_=k_d[128 * t : 128 * (t + 1), :],
                        )
                else:
                    nc.sync.dma_start(
                        out=st,
                        in_=k_d[512 * c : 512 * (c + 1), :].rearrange(
                            "(t p) k -> p t k", p=P
                        ),
                    )
                kt = ktbuf_pool.tile([P, 2, 512], F32R, tag="kt")
                for kb in range(2):
                    pt = ptrans.tile([P, 512], F32, tag="pt")
                    for t in range(4):
                        src = (
                            head[:, t, 128 * kb : 128 * (kb + 1)]
                            if use_head and t < 2
                            else st[:, t, 128 * kb : 128 * (kb + 1)]
                        )
                        nc.tensor.transpose(
                            pt[:, 128 * t : 128 * (t + 1)], src, identity
                        )
                    nc.vector.tensor_copy(kt[:, kb, :], pt)
                pp = pproj.tile([P, 512], F32, tag="pp")
                for kb in range(2):
                    nc.tensor.matmul(
                        pp,
                        lhsT=W_sb[:, kb, :],
                        rhs=kt[:, kb, :],
                        start=(kb == 0),
                        stop=(kb == 1),
                    )
                nc.scalar.activation(
                    kpT[:, 512 * c : 512 * (c + 1)],
                    pp,
                    AF.Identity,
                    bias=b_sb,
                    scale=1.0,
                )
        for v_d, ve in ((v1_d, v1e), (v2_d, v2e)):
            nc.vector.memset(ve[:, :, VD : VD + 2], 1.0)
            for c in range(nch):
                sv = stage.tile([P, 4, VD], F32, tag="stage")
                nc.sync.dma_start(
                    out=sv,
                    in_=v_d[512 * c : 512 * (c + 1), :].rearrange(
                        "(t p) d -> p t d", p=P
                    ),
                )
                nc.any.tensor_copy(ve[:, 4 * c : 4 * (c + 1), 0:VD], sv)

    # ---- Phase 2: score matmuls + exp (both orientations) ----
    # ---- Phase 3: output matmuls with folded softmax denominators ----
    with tc.tile_pool(name="pscore", bufs=2, space="PSUM") as pscore, (
        tc.tile_pool(name="po", bufs=2, space="PSUM")
    ) as po_pool, tc.tile_pool(
        name="ptet", bufs=2, space="PSUM"
    ) as ptet, tc.tile_pool(name="osb", bufs=4) as osb_pool, tc.tile_pool(
        name="rc", bufs=4
    ) as rc_pool:
        w = min(1024, n)
        tg = min(4, nt)  # E-tiles per ET-transpose burst / psum tile
        for i in range(nt):
            for h in range(n // w):
                ps = pscore.tile([P, w], F32, tag="ps")
                for q in range(w // 512):
                    col = w * h + 512 * q
                    nc.tensor.matmul(
                        ps[:, 512 * q : 512 * (q + 1)],
                        lhsT=k1pT[:, 128 * i : 128 * (i + 1)],
                        rhs=k2pT[:, col : col + 512],
                        start=True,
                        stop=True,
                    )
                nc.scalar.activation(E[:, i, w * h : w * (h + 1)], ps, AF.Exp)
            if i % tg == tg - 1:
                # ET[:, j, 128 i':128(i'+1)] = E[:, i', 128 j:128(j+1)]^T for the
                # tg E-tiles just produced — emitted here so the scheduler can
                # fill PE idle slots while ACT drains the score exps.
                g = i // tg
                for j in range(nt):
                    pt = ptet.tile([P, 128 * tg], BF16, tag="ptet")
                    for t in range(tg):
                        nc.tensor.transpose(
                            pt[:, 128 * t : 128 * (t + 1)],
                            E[:, tg * g + t, 128 * j : 128 * (j + 1)],
                            id_bf,
                        )
                    nc.any.tensor_copy(
                        ET[:, j, 128 * tg * g : 128 * tg * (g + 1)], pt
                    )

        for mt in range(nt):
            _emit_o_group(nc, po_pool, rc_pool, osb_pool, E, v1e, o1_d, mt, nt)
        for mt in range(nt):
            _emit_o_group(nc, po_pool, rc_pool, osb_pool, ET, v2e, o2_d, mt, nt)


def build_nc(n: int = N, reps: int = 1):
    """Build the single-core SPMD program. `n` is the row count (2048 on HW;
    smaller for simulator checks). `reps` repeats the body for timing runs."""
    nc = bacc.Bacc("TRN2", target_bir_lowering=False, debug=False)

    dram = {
        "k1": nc.dram_tensor("k1", [n, KD], F32, kind="ExternalInput").ap(),
        "k2": nc.dram_tensor("k2", [n, KD], F32, kind="ExternalInput").ap(),
        "v1": nc.dram_tensor("v1", [n, VD], F32, kind="ExternalInput").ap(),
        "v2": nc.dram_tensor("v2", [n, VD], F32, kind="ExternalInput").ap(),
        "o1": nc.dram_tensor("o1", [n, VD], F32, kind="ExternalOutput").ap(),
        "o2": nc.dram_tensor("o2", [n, VD], F32, kind="ExternalOutput").ap(),
    }
    W1_d = nc.dram_tensor("W1", [KD, AD], F32R, kind="ExternalInput").ap()
    b1_d = nc.dram_tensor("b1", [AD], F32, kind="ExternalInput").ap()
    W2_d = nc.dram_tensor("W2", [KD, AD], F32R, kind="ExternalInput").ap()
    b2_d = nc.dram_tensor("b2", [AD], F32, kind="ExternalInput").ap()

    with tile.TileContext(nc) as tc:
        with tc.tile_pool(name="consts", bufs=1) as consts_pool, tc.tile_pool(
            name="persist", bufs=1
        ) as persist:
            identity = consts_pool.tile([P, P], F32)
            make_identity(nc, identity)
            id_bf = consts_pool.tile([P, P], BF16)
            make_identity(nc, id_bf)
            # head: first k1 tile lands before the (later-needed) const DMAs
            head = consts_pool.tile([P, 2, KD], F32)
            for t in range(2):
                nc.sync.dma_start(
                    out=head[:, t, :],
                    in_=dram["k1"][128 * t : 128 * (t + 1), :],
                )
            # consts go via SWDGE (gpsimd) so they don't serialize ahead of
            # the first k-tile loads on the sync sequencer
            W1_sb = consts_pool.tile([P, 2, AD], F32R)
            nc.gpsimd.dma_start(out=W1_sb, in_=W1_d.rearrange("(kb k) a -> k kb a", k=P))
            W2_sb = consts_pool.tile([P, 2, AD], F32R)
            nc.gpsimd.dma_start(out=W2_sb, in_=W2_d.rearrange("(kb k) a -> k kb a", k=P))
            b1_sb = consts_pool.tile([P, 1], F32)
            nc.gpsimd.dma_start(out=b1_sb, in_=b1_d.rearrange("(a one) -> a one", one=1))
            b2_sb = consts_pool.tile([P, 1], F32)
            nc.gpsimd.dma_start(out=b2_sb, in_=b2_d.rearrange("(a one) -> a one", one=1))
            consts = (identity, id_bf, W1_sb, b1_sb, W2_sb, b2_sb)

            for _ in range(reps):
                _emit_body(nc, tc, consts, persist, dram, n, head=head)

    nc.compile()
    return nc


_NC_CACHE: dict = {}


def _get_nc(n: int = N):
    if n not in _NC_CACHE:
        _NC_CACHE[n] = build_nc(n)
    return _NC_CACHE[n]


def kernel(k1, k2, v1, v2, W1, b1, W2, b2):
    """Full-input entry point: shard batch across 8 cores, run SPMD, gather."""
    nc = _get_nc(N)
    k1 = np.ascontiguousarray(np.asarray(k1, dtype=np.float32))
    k2 = np.ascontiguousarray(np.asarray(k2, dtype=np.float32))
    v1 = np.ascontiguousarray(np.asarray(v1, dtype=np.float32))
    v2 = np.ascontiguousarray(np.asarray(v2, dtype=np.float32))
    W1 = np.ascontiguousarray(np.asarray(W1, dtype=np.float32))
    b1 = np.ascontiguousarray(np.asarray(b1, dtype=np.float32))
    W2 = np.ascontiguousarray(np.asarray(W2, dtype=np.float32))
    b2 = np.ascontiguousarray(np.asarray(b2, dtype=np.float32))
    in_maps = [
        {
            "k1": k1[c],
            "k2": k2[c],
            "v1": v1[c],
            "v2": v2[c],
            "W1": W1,
            "b1": b1,
            "W2": W2,
            "b2": b2,
        }
        for c in range(N_CORES)
    ]
    res = bass_utils.run_bass_kernel_spmd(nc, in_maps, core_ids=list(range(N_CORES)))
    o2 = np.stack([res.results[c]["o2"] for c in range(N_CORES)])
    o1 = np.stack([res.results[c]["o1"] for c in range(N_CORES)])
    return (o2, o1)
